# revision 21
# baseline (speedup 1.0000x reference)
"""Trainium2 Bass kernel for nn_Attention_3375844295015.

RMSNorm -> {Q (normalized), KV (unnormalized)} projections -> RoPE(q,k)
-> causal attention -> out projection, distributed over 8 NeuronCores
Megatron-style: each core owns 2 of the 16 heads (column-shard of
Wq/Wk/Wv, row-shard of Wo) and produces a full-shape partial output;
the host sums the 8 partials (the "all-reduce") and adds bo.

Per-core dataflow (transposed [feature, seq] layout so matmuls chain
without transposes):
  qT = rope(Wq_c^T @ xT) * r * scale   (r = RMSNorm scale, folded into
                                        the rope cos/sin tables host-side)
  kT = rope(Wk_c^T @ xT)
  vT = Wv_c^T @ xT, then PE-transposed to v [kpos, dim] blocks with 64
       replicated ones-columns appended (so A@V accumulates Sum(e*v) on
       pav rows 0:64 and Sum(e) replicated on rows 64:128)
  per (batch, head): simT[k, q] = kT^T qTpad (qT zero-padded per head so
  K=128); e = exp(simT) (no max-sub: logits are O(+-10)); causal mask via
  0/1 multiply restricted to the 128-col diagonal window; AV accumulates
  in PSUM; normalization = exp(-ln(sum)) on scalar times pav[0:64] on
  vector; out projection = outT^T @ Wo_c per 128-token block.

Engine balance (v2): scalar = exp/ln only; vector = rope muls/adds, mask,
tail muls, vtsb/vb copies; gpsimd = all PSUM->SBUF casts (projection raw,
out-projection), memsets, DMA triggers. DMA layouts are arranged so every
big transfer moves >=8KB contiguous per partition (xt is staged per
512-token band as [128, 8, 512]).
"""

import sys

sys.path.insert(0, "/opt/trn_rl_repo")

import numpy as np
import ml_dtypes

import concourse.bass as bass
import concourse.mybir as mybir
import concourse.tile as tile_mod
from concourse.bass_utils import run_bass_kernel_spmd
from concourse.vector_clock import ScopedClock

BF16 = ml_dtypes.bfloat16
F32 = mybir.dt.float32
BF = mybir.dt.bfloat16
AF = mybir.ActivationFunctionType

B, N, D = 2, 2048, 1024
H, DH = 16, 64
INNER = H * DH
EPS = 1e-8
SCALE = DH ** -0.5
NCORES = 8
BN = B * N              # 4096 tokens, col index = b*N + n
KC = 128                # k-position chunk
QT = 512                # q-tile width
NQT_B = N // QT         # 4 q-tiles per batch


def _patched_drain_and_barrier(self, tick_clock, wait_clock):
    # The stock TileContext drain carries one sem-wait per outstanding
    # logical processor; this neuronxcc lowers SP Drain through a CTRL
    # struct that holds fewer waits ("Too many sync wait commands").
    # Put each wait on its own SP NOP ahead of the drain instead.
    nop_inst = self.nc.sync.nop(nofuse=True, hint="pre_drain_waits")
    wait_clock.add_sem_waits(
        nop_inst.ins, ScopedClock({None: tick_clock.global_clock})
    )
    si = nop_inst.ins.sync_info
    waits = list(si.on_wait) if si is not None else []
    if len(waits) > 1:
        si.on_wait = waits[:1]
        for w in waits[1:]:
            extra = self.nc.sync.nop(nofuse=True, hint="pre_drain_waits")
            extra.ins.sync_info = mybir.SyncInfo(on_wait=[w], on_update=[])
    self.nc.sync.drain()
    self.nc.all_engine_barrier()
    popped = self.nc._tile_sem_poison_stack.pop()
    assert popped is self._sem_poison
    self.nc.clear_and_free_semaphores(list(self.sems.allocated().values()))


tile_mod.TileContext._drain_and_barrier = _patched_drain_and_barrier


def _split_excess_waits(nc, limit=1):
    """walrus CoreV3 lowers at most ~1 sem wait per instruction; move any
    excess onto same-engine NOPs inserted directly before the instruction
    (same-engine program order makes this semantically identical)."""
    ctr = [0]
    for f in nc.m.functions:
        for bb in f.blocks:
            new_insts = []
            for inst in bb.instructions:
                si = inst.sync_info
                lim = 1 if type(inst).__name__ == "InstDrain" else limit
                if si is not None and len(si.on_wait) > lim:
                    waits = list(si.on_wait)
                    si.on_wait = waits[-lim:]
                    extra = waits[:-lim]
                    for i in range(0, len(extra), limit):
                        ctr[0] += 1
                        nop = mybir.InstNoOp(
                            name=f"WSPLIT-{ctr[0]}",
                            engine=inst.engine,
                            bass_nofuse=True,
                            sync_info=mybir.SyncInfo(
                                on_wait=extra[i:i + limit], on_update=[]
                            ),
                        )
                        new_insts.append(nop)
                new_insts.append(inst)
            bb.instructions[:] = new_insts
    return ctr[0]


def _build_program():
    nc = bass.Bass()
    dt = mybir.dt

    # xt staged per 512-token band: [jband, part, kc, col] so one DMA per
    # band moves 8KB contiguous per partition.
    xt_d = nc.declare_dram_parameter("xt", [8, 128, 8, QT], dt.bfloat16, isOutput=False)
    wq_d = nc.declare_dram_parameter("wq", [128, 1024], dt.bfloat16, isOutput=False)
    wk_d = nc.declare_dram_parameter("wk", [128, 1024], dt.bfloat16, isOutput=False)
    wv_d = nc.declare_dram_parameter("wv", [128, 1024], dt.bfloat16, isOutput=False)
    wo_d = nc.declare_dram_parameter("wo", [128, 1024], dt.bfloat16, isOutput=False)
    cosq_d = nc.declare_dram_parameter("cosq", [128, BN], dt.bfloat16, isOutput=False)
    sinq_d = nc.declare_dram_parameter("sinq", [128, BN], dt.bfloat16, isOutput=False)
    cosk_d = nc.declare_dram_parameter("cosk", [128, N], dt.bfloat16, isOutput=False)
    sink_d = nc.declare_dram_parameter("sink", [128, N], dt.bfloat16, isOutput=False)
    iden_d = nc.declare_dram_parameter("iden", [128, 128], dt.bfloat16, isOutput=False)
    tri_d = nc.declare_dram_parameter("tri", [128, 128], dt.bfloat16, isOutput=False)
    out_d = nc.declare_dram_parameter("out", [BN, D], dt.bfloat16, isOutput=True)

    from contextlib import ExitStack

    with tile_mod.TileContext(nc) as tc, ExitStack() as ctx:
        consts = ctx.enter_context(tc.tile_pool(name="consts", bufs=1))
        sbuf = ctx.enter_context(tc.tile_pool(name="sbuf", bufs=1))
        work = ctx.enter_context(tc.tile_pool(name="work", bufs=4))
        rawp = ctx.enter_context(tc.tile_pool(name="rawp", bufs=3))
        epool = ctx.enter_context(tc.tile_pool(name="epool", bufs=6))
        rpool = ctx.enter_context(tc.tile_pool(name="rpool", bufs=4))
        ps_sim = ctx.enter_context(tc.tile_pool(name="ps_sim", bufs=2, space="PSUM"))
        ps_av = ctx.enter_context(tc.tile_pool(name="ps_av", bufs=2, space="PSUM"))
        ps_fill = ctx.enter_context(tc.tile_pool(name="ps_fill", bufs=2, space="PSUM"))

        wq_sb = consts.tile([128, 8, 128], BF, tag="wq")
        xt_sb = consts.tile([128, 8, 8, QT], BF, tag="xt")      # (jb, kc, col)
        wk_sb = consts.tile([128, 8, 128], BF, tag="wk")
        wv_sb = consts.tile([128, 8, 128], BF, tag="wv")
        iden_sb = consts.tile([128, 128], BF, tag="iden")
        tri_sb = consts.tile([128, 128], BF, tag="tri")
        cosq_sb = consts.tile([128, BN], BF, tag="cosq")
        sinq_sb = consts.tile([128, BN], BF, tag="sinq")
        cosk_sb = consts.tile([128, N], BF, tag="cosk")
        sink_sb = consts.tile([128, N], BF, tag="sink")
        wo_sb = consts.tile([128, 1024], BF, tag="wo")

        # ---- persistent intermediates (memsets BEFORE any DMA trigger on
        # gpsimd: zero/ones planes gate the first sims and AVs, and the
        # gpsimd queue would otherwise run them after its DMA triggers) ----
        qTpad = sbuf.tile([128, 2, BN], BF, tag="qTpad")
        kT = sbuf.tile([128, BN], BF, tag="kT")
        # v blocks [kpos, head, 64 dims | 64 ones]; ones columns make AV
        # emit Sum(e) replicated on pav rows 64:128.
        vb = sbuf.tile([128, 32, 2, 128], BF, tag="vb")
        outT = sbuf.tile([128, BN], BF, tag="outT")
        nc.gpsimd.memset(qTpad[64:128, 0, :], 0.0)
        nc.gpsimd.memset(qTpad[0:64, 1, :], 0.0)
        nc.gpsimd.memset(vb[:, 0:16, 0, 64:128], 1.0)
        nc.gpsimd.memset(vb[:, 0:16, 1, 64:128], 1.0)
        nc.gpsimd.memset(vb[:, 16:32, 0, 64:128], 1.0)
        nc.gpsimd.memset(vb[:, 16:32, 1, 64:128], 1.0)

        def xt_band(eng, j):
            eng.dma_start(xt_sb[:, j], xt_d[j])

        # ---- DMA schedule. Big descriptors; rings drain roughly in
        # trigger order, so issue strictly by need-time. sync carries the
        # critical path (wq -> xt0 -> wk -> xt bands); scalar carries the
        # small urgent tables; gpsimd (busy with memsets first) gets the
        # late tables.
        nc.sync.dma_start(wq_sb[:], wq_d[:].rearrange("p (k m) -> p k m", k=8))
        nc.sync.dma_start(xt_sb[:, 0, 0:4], xt_d[0][:, 0:4])
        nc.sync.dma_start(xt_sb[:, 0, 4:8], xt_d[0][:, 4:8])
        nc.sync.dma_start(wk_sb[:], wk_d[:].rearrange("p (k m) -> p k m", k=8))
        xt_band(nc.sync, 1)
        xt_band(nc.sync, 2)
        xt_band(nc.sync, 3)
        xt_band(nc.sync, 4)
        xt_band(nc.sync, 5)
        xt_band(nc.sync, 6)
        xt_band(nc.sync, 7)

        nc.scalar.dma_start(cosq_sb[:, 0:QT], cosq_d[:, 0:QT])
        nc.scalar.dma_start(sinq_sb[:, 0:QT], sinq_d[:, 0:QT])
        nc.scalar.dma_start(wv_sb[:], wv_d[:].rearrange("p (k m) -> p k m", k=8))
        nc.scalar.dma_start(cosk_sb[:, 0:QT], cosk_d[:, 0:QT])
        nc.scalar.dma_start(sink_sb[:, 0:QT], sink_d[:, 0:QT])
        nc.scalar.dma_start(iden_sb[:], iden_d[:])
        nc.scalar.dma_start(tri_sb[:], tri_d[:])
        nc.scalar.dma_start(cosq_sb[:, QT:1024], cosq_d[:, QT:1024])
        nc.scalar.dma_start(sinq_sb[:, QT:1024], sinq_d[:, QT:1024])
        nc.scalar.dma_start(cosk_sb[:, QT:1024], cosk_d[:, QT:1024])
        nc.scalar.dma_start(sink_sb[:, QT:1024], sink_d[:, QT:1024])
        nc.scalar.dma_start(wo_sb[:], wo_d[:])

        nc.gpsimd.dma_start(cosq_sb[:, 1024:2048], cosq_d[:, 1024:2048])
        nc.gpsimd.dma_start(sinq_sb[:, 1024:2048], sinq_d[:, 1024:2048])
        nc.gpsimd.dma_start(cosk_sb[:, 1024:2048], cosk_d[:, 1024:2048])
        nc.gpsimd.dma_start(sink_sb[:, 1024:2048], sink_d[:, 1024:2048])
        nc.gpsimd.dma_start(cosq_sb[:, 2048:3072], cosq_d[:, 2048:3072])
        nc.gpsimd.dma_start(sinq_sb[:, 2048:3072], sinq_d[:, 2048:3072])
        nc.gpsimd.dma_start(cosq_sb[:, 3072:4096], cosq_d[:, 3072:4096])
        nc.gpsimd.dma_start(sinq_sb[:, 3072:4096], sinq_d[:, 3072:4096])

        # ---- emission units ----

        def proj_chain(w_sb, j):
            """8-step accumulation chain for band j into one 1-bank slot."""
            holder = {}

            def step(kc, holder=holder):
                if kc == 0:
                    holder["raw"] = ps_fill.tile(
                        [128, QT], F32, tag="fill", name="chain"
                    )
                nc.tensor.matmul(
                    holder["raw"],
                    w_sb[:, kc, :],
                    xt_sb[:, j, kc, :],
                    start=(kc == 0),
                    stop=(kc == 7),
                )

            return holder, [(1, lambda kc=kc: step(kc)) for kc in range(8)]

        # head dims are host-permuted into interleaved rotate-half pairs
        # (2f <- f, 2f+1 <- f+32), so rotate_half is an adjacent-pair
        # partition swap: one DVE stream_shuffle
        SWAP_MASK = [i ^ 1 for i in range(32)]

        def rope_unit(holder, j, cos_sb, sin_sb, is_q):
            """cast raw (PSUM f32) to SBUF bf16, then rope via shuffle +
            two bf16 muls + add, all at 2x DVE rate."""
            jc = slice(j * QT, (j + 1) * QT)
            jq = j % NQT_B
            tc_ = jc if is_q else slice(jq * QT, (jq + 1) * QT)

            def cast():
                raws = rawp.tile([128, QT], BF, tag="raws", name="raws")
                nc.vector.tensor_copy(raws[:], holder["raw"])
                holder["raws"] = raws

            def run():
                raws = holder["raws"]
                t2r = work.tile([128, QT], BF, tag="t2r")
                nc.vector.stream_shuffle(t2r[:], raws[:], SWAP_MASK)
                t1 = work.tile([128, QT], BF, tag="t1")
                nc.vector.tensor_mul(t1[:], raws[:], cos_sb[:, tc_])
                t2 = work.tile([128, QT], BF, tag="t2")
                nc.vector.tensor_mul(t2[:], t2r[:], sin_sb[:, tc_])
                if is_q:
                    nc.vector.tensor_add(
                        qTpad[0:64, 0, jc], t1[0:64, :], t2[0:64, :]
                    )
                    nc.vector.tensor_add(
                        qTpad[64:128, 1, jc], t1[64:128, :], t2[64:128, :]
                    )
                else:
                    nc.vector.tensor_add(kT[:, jc], t1[:], t2[:])

            return [(0, cast), (0, run)]

        def vt_units(j):
            """vT chain + transpose/scatter into vb for band j."""
            holder, steps = proj_chain(wv_sb, j)
            units = list(steps)

            def copyout():
                vtsb = work.tile([128, QT], BF, tag="vtsb", bufs=2)
                nc.vector.tensor_copy(vtsb[:], holder["raw"])
                holder["vtsb"] = vtsb

            units.append((0, copyout))

            def tr(u, j=j):
                vtsb = holder["vtsb"]
                tp = ps_fill.tile([128, 128], BF, tag="fill", name="tp")
                nc.tensor.transpose(
                    tp[:, 0:128],
                    vtsb[:, u * 128:(u + 1) * 128],
                    iden_sb[:],
                )
                rc = j * 4 + u
                nc.vector.tensor_copy(
                    vb[:, rc, :, 0:64],
                    tp[:].rearrange("p (h d) -> p h d", h=2),
                )

            units.extend((1, lambda u=u: tr(u)) for u in range(4))
            return units

        def j_units(j):
            """All projection work for band j. The PSUM->SBUF casts are
            emitted immediately after their chain (ahead of the rope
            arithmetic) so the fill slots recycle fast and the next chain
            never waits on the vector queue's rope backlog."""
            qh, qsteps = proj_chain(wq_sb, j)
            cast_q, run_q = rope_unit(qh, j, cosq_sb, sinq_sb, True)
            kh, ksteps = proj_chain(wk_sb, j)
            cast_k, run_k = rope_unit(kh, j, cosk_sb, sink_sb, False)
            vsteps = vt_units(j)
            units = []
            units.extend(qsteps)
            units.append(cast_q)
            units.extend(ksteps)
            units.append(cast_k)
            units.append(run_q)
            units.extend(vsteps[:9])       # v chain + copyout
            units.append(run_k)
            units.extend(vsteps[9:])       # transposes
            return units

        def outproj(m, endgame=False):
            mrows = slice(m * 128, (m + 1) * 128)
            # endgame blocks (after the last AV) borrow the now-dead
            # sim/av PSUM slots so 3 blocks can be in flight instead of 1
            if endgame:
                # NOT the "av" slots - pav is still being read by the
                # remaining tail chunks
                po0 = ps_sim.tile([128, QT], F32, tag="sim", name="po0")
                po1 = ps_fill.tile([128, QT], F32, tag="fill", name="po1")
            else:
                po0 = ps_fill.tile([128, QT], F32, tag="fill", name="po0")
                po1 = ps_fill.tile([128, QT], F32, tag="fill", name="po1")
            nc.tensor.matmul(
                po0[:], outT[:, mrows], wo_sb[:, 0:QT], start=True, stop=True
            )
            nc.tensor.matmul(
                po1[:], outT[:, mrows], wo_sb[:, QT:1024], start=True, stop=True
            )
            # stage to SBUF bf16 (PSUM is not DMA-able). During attention
            # the scalar engine is saturated with exp, so casts go to
            # vector; once attention winds down (m >= 24) scalar takes one
            # of the pair so the two casts run in parallel.
            ot = work.tile([128, 1024], BF, tag="ot")
            if m >= 24:
                nc.scalar.copy(ot[:, 0:QT], po0[:])
            else:
                nc.vector.tensor_copy(ot[:, 0:QT], po0[:])
            nc.vector.tensor_copy(ot[:, QT:1024], po1[:])
            oeng = nc.gpsimd if m % 2 == 0 and m < 24 else nc.sync
            oeng.dma_start(out_d[mrows, :], ot[:])

        def attn_units(b):
            """Yields ("kc", closure) and ("tail", t, closure)."""
            base = b * N
            for t in range(NQT_B):
                qcols = slice(base + t * QT, base + (t + 1) * QT)
                nkc = 4 * (t + 1)
                pav = [None, None]
                yield ("gate", 4 * b + t)

                pend_av = [None]

                def emit_av(kc, e, lo, b=b, pav=pav, nkc=nkc):
                    if kc == 0:
                        pav[0] = ps_av.tile([128, QT], F32, tag="av", name="pav0")
                        pav[1] = ps_av.tile([128, QT], F32, tag="av", name="pav1")
                    for h in range(2):
                        nc.tensor.matmul(
                            pav[h][:, lo:],
                            vb[:, b * 16 + kc, h, :],
                            e[:, h, lo:],
                            start=(kc == 0),
                            stop=(kc == nkc - 1),
                        )

                def kc_block(kc, t=t, b=b, base=base, pend_av=pend_av):
                    off = kc * KC - t * QT
                    lo = max(0, off)
                    kcols = slice(base + kc * KC, base + (kc + 1) * KC)
                    qsub = slice(base + t * QT + lo, base + (t + 1) * QT)
                    ps_s = ps_sim.tile([128, 2, QT], F32, tag="sim", name="sim")
                    for h in range(2):
                        nc.tensor.matmul(
                            ps_s[:, h, lo:],
                            kT[:, kcols],
                            qTpad[:, h, qsub],
                            start=True,
                            stop=True,
                        )
                    e = epool.tile([128, 2, QT], BF, tag="e", name="e")
                    nc.scalar.activation(e[:, :, lo:], ps_s[:, :, lo:], AF.Exp)
                    if off >= 0:
                        # only the 128-col diagonal window is mixed; the
                        # rest of the row is fully unmasked
                        hi = lo + KC
                        nc.gpsimd.tensor_mul(
                            e[:, 0, lo:hi], e[:, 0, lo:hi], tri_sb[:]
                        )
                        nc.gpsimd.tensor_mul(
                            e[:, 1, lo:hi], e[:, 1, lo:hi], tri_sb[:]
                        )
                    # software pipeline: emit the PREVIOUS kc's AV here so
                    # this kc's sims sit between exp(kc-1) and AV(kc-1) in
                    # the PE queue, hiding the activation latency
                    if pend_av[0] is not None:
                        emit_av(*pend_av[0])
                    pend_av[0] = (kc, e, lo)

                def flush_av(pend_av=pend_av):
                    emit_av(*pend_av[0])
                    pend_av[0] = None

                for kc in range(nkc):
                    yield ("kc", lambda kc=kc, f=kc_block: f(kc))
                yield ("kc", lambda: flush_av())

                last = (b == 1 and t == NQT_B - 1)

                def tail_chunk(c, nchunks, qcols=qcols, pav=pav):
                    cw = QT // nchunks
                    cs = slice(c * cw, (c + 1) * cw)
                    ocs = slice(qcols.start + c * cw, qcols.start + (c + 1) * cw)
                    for h in range(2):
                        hp = slice(64 * h, 64 * h + 64)
                        lnt = rpool.tile([64, cw], F32, tag="lnt", name="lnt")
                        nc.scalar.activation(lnt[:], pav[h][64:128, cs], AF.Ln)
                        rec = rpool.tile([64, cw], BF, tag="rec", name="rec")
                        nc.scalar.activation(rec[:], lnt[:], AF.Exp, scale=-1.0)
                        nc.vector.tensor_mul(
                            outT[hp, ocs], pav[h][0:64, cs], rec[:]
                        )

                yield ("tail", b, t, last, tail_chunk)

        # ---- prologue: band 0 only, then one unified schedule:
        # attention qtiles (both batches, in order) interleaved with the
        # remaining per-j projection units (need-ordered, matching DMA
        # arrival) and, as soon as each qtile's tail lands, its
        # out-projection blocks. ----
        for _, f in j_units(0):
            f()

        pending = []
        j_end = {0: 0}
        for j in range(1, 8):
            pending.extend(j_units(j))
            j_end[j] = len(pending)
        fi = 0

        def attn_all():
            yield from attn_units(0)
            yield from attn_units(1)

        gate_now = [0]
        for item in attn_all():
            if item[0] == "gate":
                # qtile (b,t) reads qTpad/kT/vb written by j-units up to
                # j = 4b+t; those must be EMITTED before the qtile's
                # instructions or the reads get no dependency edge
                gate_now[0] = item[1]
                need = j_end[item[1]]
                while fi < need:
                    pending[fi][1]()
                    fi += 1
            elif item[0] == "kc":
                item[1]()
                # hold back out-projection work (appended past nproj) so
                # the filler-starved final qtiles stay PE-dense: b1t3
                # (gate 7) gets everything, b1t2 keeps 12 pe-units in
                # reserve, earlier tiles keep 16
                if gate_now[0] >= 7:
                    limit = len(pending)
                elif gate_now[0] == 6:
                    limit = len(pending) - 6
                else:
                    limit = max(
                        j_end[7], fi + max(0, len(pending) - fi - 16)
                    )
                budget = 2
                while fi < min(limit, len(pending)) and (
                    budget > 0 or pending[fi][0] == 0
                ):
                    pe, f = pending[fi]
                    f()
                    budget -= pe
                    fi += 1
            else:
                _, b_, t_, last_, tailf = item
                if last_:
                    # final qtile: pipeline each 128-token chunk's
                    # normalization straight into its out-projection so
                    # the last DMAs start as early as possible; drain any
                    # held-back blocks between chunks to keep the PE warm
                    m0 = 16 * b_ + 4 * t_
                    for c in range(4):
                        tailf(c, 4)
                        outproj(m0 + c, endgame=True)
                        budget = 2
                        while fi < len(pending) and budget > 0:
                            pe, f = pending[fi]
                            f()
                            budget -= pe
                            fi += 1
                else:
                    tailf(0, 1)
                    # filler burst: give the PE queue extra work to chew
                    # on while the tail chain releases the AV banks (but
                    # respect the endgame hold-back)
                    budget = 3
                    hold = 6 if gate_now[0] >= 6 else 16
                    while fi < len(pending) - hold and budget > 0:
                        pe, f = pending[fi]
                        f()
                        budget -= pe
                        fi += 1
                    for m in range(16 * b_ + 4 * t_, 16 * b_ + 4 * t_ + 4):
                        pending.append((2, lambda m=m: outproj(m)))
        while fi < len(pending):
            pending[fi][1]()
            fi += 1

    _split_excess_waits(nc)
    return nc


_PROGRAM = None


def _get_program():
    global _PROGRAM
    if _PROGRAM is None:
        _PROGRAM = _build_program()
    return _PROGRAM


def _host_prep(x, pos_emb, gamma, Wq, Wkv, Wo):
    """Build the per-core input maps."""
    xf = np.ascontiguousarray(x.reshape(BN, D))
    xT = np.ascontiguousarray(xf.T).astype(BF16)        # [1024, 4096]
    # [jband, part, kc, col]: band j holds tokens j*512:(j+1)*512 for all
    # 8 feature chunks -> per-partition 8KB contiguous DMA lines
    xt = np.ascontiguousarray(
        xT.reshape(8, 128, 8, QT).transpose(2, 1, 0, 3)
    )

    r = 1.0 / np.maximum(
        np.linalg.norm(xf.astype(np.float64), axis=1).astype(np.float32)
        * (D ** -0.5),
        EPS,
    )

    fr = pos_emb[0, 0, :, :32].astype(np.float32)        # [N, 32]
    cos_t = np.cos(fr).T                                 # [32, N]
    sin_t = np.sin(fr).T
    # interleaved-pair head-dim order: partition p holds old dim
    # perm64[p%64] of its head; pair (2f, 2f+1) <- old (f, f+32).
    # rope there: out = t*cos + swap_adj(t)*sin with sign -1 on even p.
    pmod = np.arange(128) % 64
    fidx = pmod // 2                                     # freq index per row
    cos128 = cos_t[fidx, :].astype(np.float32)           # [128, N]
    sgn = np.where(np.arange(128) % 2 == 0, -1.0, 1.0).astype(np.float32)
    sin128s = (sin_t[fidx, :] * sgn[:, None]).astype(np.float32)
    colpos = np.arange(BN) % N
    rq = (r * SCALE).astype(np.float32)
    cosq = (cos128[:, colpos] * rq[None, :]).astype(BF16)
    sinq = (sin128s[:, colpos] * rq[None, :]).astype(BF16)
    cosk = cos128.astype(BF16)
    sink = sin128s.astype(BF16)
    # old-dim index for interleaved position d (within a 64-dim head)
    perm64 = np.empty(64, np.int64)
    perm64[0::2] = np.arange(32)
    perm64[1::2] = np.arange(32) + 32

    iden = np.eye(128, dtype=np.float32).astype(BF16)

    # causal 0/1 triangle for the 128-col diagonal window: keep iff p <= c
    p = np.arange(128)[:, None]
    c = np.arange(128)[None, :]
    tri = (p <= c).astype(BF16)

    def permute_heads(w):  # apply perm64 within each 64-col head block
        wh = w.reshape(w.shape[0], H, 64)
        return wh[:, :, perm64].reshape(w.shape[0], INNER)

    Wq_s = permute_heads(
        (gamma[:, None].astype(np.float32) * Wq)
    ).astype(BF16)
    Wk = permute_heads(Wkv[:, :INNER].astype(np.float32)).astype(BF16)
    Wv = Wkv[:, INNER:].astype(BF16)
    Wo_b = Wo.astype(BF16)

    def warrange(w):  # [1024, 128] -> [128, 1024] with [p, kc*128+m]
        return np.ascontiguousarray(
            w.reshape(8, 128, 128).transpose(1, 0, 2).reshape(128, 1024)
        )

    in_maps = []
    for c_ in range(NCORES):
        sl = slice(128 * c_, 128 * (c_ + 1))
        in_maps.append(
            {
                "xt": xt,
                "wq": warrange(Wq_s[:, sl]),
                "wk": warrange(Wk[:, sl]),
                "wv": warrange(Wv[:, sl]),
                "wo": np.ascontiguousarray(Wo_b[sl, :]),
                "cosq": cosq,
                "sinq": sinq,
                "cosk": cosk,
                "sink": sink,
                "iden": iden,
                "tri": tri,
            }
        )
    return in_maps


def run(inputs, trace=False, trace_kwargs=None):
    nc = _get_program()
    in_maps = _host_prep(
        np.asarray(inputs["x"]),
        np.asarray(inputs["pos_emb"]),
        np.asarray(inputs["gamma"]),
        np.asarray(inputs["Wq"]),
        np.asarray(inputs["Wkv"]),
        np.asarray(inputs["Wo"]),
    )
    res = run_bass_kernel_spmd(
        nc,
        in_maps,
        list(range(NCORES)),
        trace=trace,
        trace_kwargs=trace_kwargs or {},
    )
    out = np.zeros((BN, D), np.float32)
    for c in range(NCORES):
        out += res.results[c]["out"].astype(np.float32)
    out += np.asarray(inputs["bo"]).astype(np.float32)[None, :]
    out = out.reshape(B, N, D).astype(np.float32)
    return out, res


def kernel(**inputs):
    out, _ = run(inputs, trace=False)
    return out


# revision 23
# speedup vs baseline: 1.0463x; 1.0463x over previous
"""Trainium2 Bass kernel for nn_Attention_3375844295015.

RMSNorm -> {Q (normalized), KV (unnormalized)} projections -> RoPE(q,k)
-> causal attention -> out projection, distributed over 8 NeuronCores
Megatron-style: each core owns 2 of the 16 heads (column-shard of
Wq/Wk/Wv, row-shard of Wo) and produces a full-shape partial output;
the host sums the 8 partials (the "all-reduce") and adds bo.

Per-core dataflow (transposed [feature, seq] layout so matmuls chain
without transposes):
  qT = rope(Wq_c^T @ xT) * r * scale   (r = RMSNorm scale, folded into
                                        the rope cos/sin tables host-side)
  kT = rope(Wk_c^T @ xT)
  vT = Wv_c^T @ xT, then PE-transposed to v [kpos, dim] blocks with 64
       replicated ones-columns appended (so A@V accumulates Sum(e*v) on
       pav rows 0:64 and Sum(e) replicated on rows 64:128)
  per (batch, head): simT[k, q] = kT^T qTpad (qT zero-padded per head so
  K=128); e = exp(simT) (no max-sub: logits are O(+-10)); causal mask via
  0/1 multiply restricted to the 128-col diagonal window; AV accumulates
  in PSUM; normalization = exp(-ln(sum)) on scalar times pav[0:64] on
  vector; out projection = outT^T @ Wo_c per 128-token block.

Engine balance (v2): scalar = exp/ln only; vector = rope muls/adds, mask,
tail muls, vtsb/vb copies; gpsimd = all PSUM->SBUF casts (projection raw,
out-projection), memsets, DMA triggers. DMA layouts are arranged so every
big transfer moves >=8KB contiguous per partition (xt is staged per
512-token band as [128, 8, 512]).
"""

import sys

sys.path.insert(0, "/opt/trn_rl_repo")

import numpy as np
import ml_dtypes

import concourse.bass as bass
import concourse.mybir as mybir
import concourse.tile as tile_mod
from concourse.bass_utils import run_bass_kernel_spmd
from concourse.vector_clock import ScopedClock

BF16 = ml_dtypes.bfloat16
F32 = mybir.dt.float32
BF = mybir.dt.bfloat16
AF = mybir.ActivationFunctionType

B, N, D = 2, 2048, 1024
H, DH = 16, 64
INNER = H * DH
EPS = 1e-8
SCALE = DH ** -0.5
NCORES = 8
BN = B * N              # 4096 tokens, col index = b*N + n
KC = 128                # k-position chunk
QT = 512                # q-tile width
NQT_B = N // QT         # 4 q-tiles per batch


def _patched_drain_and_barrier(self, tick_clock, wait_clock):
    # The stock TileContext drain carries one sem-wait per outstanding
    # logical processor; this neuronxcc lowers SP Drain through a CTRL
    # struct that holds fewer waits ("Too many sync wait commands").
    # Put each wait on its own SP NOP ahead of the drain instead.
    nop_inst = self.nc.sync.nop(nofuse=True, hint="pre_drain_waits")
    wait_clock.add_sem_waits(
        nop_inst.ins, ScopedClock({None: tick_clock.global_clock})
    )
    si = nop_inst.ins.sync_info
    waits = list(si.on_wait) if si is not None else []
    if len(waits) > 1:
        si.on_wait = waits[:1]
        for w in waits[1:]:
            extra = self.nc.sync.nop(nofuse=True, hint="pre_drain_waits")
            extra.ins.sync_info = mybir.SyncInfo(on_wait=[w], on_update=[])
    self.nc.sync.drain()
    self.nc.all_engine_barrier()
    popped = self.nc._tile_sem_poison_stack.pop()
    assert popped is self._sem_poison
    self.nc.clear_and_free_semaphores(list(self.sems.allocated().values()))


tile_mod.TileContext._drain_and_barrier = _patched_drain_and_barrier


def _split_excess_waits(nc, limit=1):
    """walrus CoreV3 lowers at most ~1 sem wait per instruction; move any
    excess onto same-engine NOPs inserted directly before the instruction
    (same-engine program order makes this semantically identical)."""
    ctr = [0]
    for f in nc.m.functions:
        for bb in f.blocks:
            new_insts = []
            for inst in bb.instructions:
                si = inst.sync_info
                lim = 1 if type(inst).__name__ == "InstDrain" else limit
                if si is not None and len(si.on_wait) > lim:
                    waits = list(si.on_wait)
                    si.on_wait = waits[-lim:]
                    extra = waits[:-lim]
                    for i in range(0, len(extra), limit):
                        ctr[0] += 1
                        nop = mybir.InstNoOp(
                            name=f"WSPLIT-{ctr[0]}",
                            engine=inst.engine,
                            bass_nofuse=True,
                            sync_info=mybir.SyncInfo(
                                on_wait=extra[i:i + limit], on_update=[]
                            ),
                        )
                        new_insts.append(nop)
                new_insts.append(inst)
            bb.instructions[:] = new_insts
    return ctr[0]


def _build_program():
    nc = bass.Bass()
    dt = mybir.dt

    # xt staged per 512-token band: [jband, part, kc, col] so one DMA per
    # band moves 8KB contiguous per partition.
    xt_d = nc.declare_dram_parameter("xt", [8, 128, 8, QT], dt.bfloat16, isOutput=False)
    wq_d = nc.declare_dram_parameter("wq", [128, 1024], dt.bfloat16, isOutput=False)
    wk_d = nc.declare_dram_parameter("wk", [128, 1024], dt.bfloat16, isOutput=False)
    wv_d = nc.declare_dram_parameter("wv", [128, 1024], dt.bfloat16, isOutput=False)
    wo_d = nc.declare_dram_parameter("wo", [128, 1024], dt.bfloat16, isOutput=False)
    cosq_d = nc.declare_dram_parameter("cosq", [128, BN], dt.bfloat16, isOutput=False)
    sinq_d = nc.declare_dram_parameter("sinq", [128, BN], dt.bfloat16, isOutput=False)
    cosk_d = nc.declare_dram_parameter("cosk", [128, N], dt.bfloat16, isOutput=False)
    sink_d = nc.declare_dram_parameter("sink", [128, N], dt.bfloat16, isOutput=False)
    iden_d = nc.declare_dram_parameter("iden", [128, 128], dt.bfloat16, isOutput=False)
    tri_d = nc.declare_dram_parameter("tri", [128, 128], dt.bfloat16, isOutput=False)
    out_d = nc.declare_dram_parameter("out", [BN, D], dt.bfloat16, isOutput=True)

    from contextlib import ExitStack

    with tile_mod.TileContext(nc) as tc, ExitStack() as ctx:
        consts = ctx.enter_context(tc.tile_pool(name="consts", bufs=1))
        sbuf = ctx.enter_context(tc.tile_pool(name="sbuf", bufs=1))
        work = ctx.enter_context(tc.tile_pool(name="work", bufs=4))
        rawp = ctx.enter_context(tc.tile_pool(name="rawp", bufs=3))
        epool = ctx.enter_context(tc.tile_pool(name="epool", bufs=6))
        rpool = ctx.enter_context(tc.tile_pool(name="rpool", bufs=4))
        ps_sim = ctx.enter_context(tc.tile_pool(name="ps_sim", bufs=2, space="PSUM"))
        ps_av = ctx.enter_context(tc.tile_pool(name="ps_av", bufs=2, space="PSUM"))
        ps_fill = ctx.enter_context(tc.tile_pool(name="ps_fill", bufs=2, space="PSUM"))

        wq_sb = consts.tile([128, 8, 128], BF, tag="wq")
        xt_sb = consts.tile([128, 8, 8, QT], BF, tag="xt")      # (jb, kc, col)
        wk_sb = consts.tile([128, 8, 128], BF, tag="wk")
        wv_sb = consts.tile([128, 8, 128], BF, tag="wv")
        iden_sb = consts.tile([128, 128], BF, tag="iden")
        tri_sb = consts.tile([128, 128], BF, tag="tri")
        cosq_sb = consts.tile([128, BN], BF, tag="cosq")
        sinq_sb = consts.tile([128, BN], BF, tag="sinq")
        cosk_sb = consts.tile([128, N], BF, tag="cosk")
        sink_sb = consts.tile([128, N], BF, tag="sink")
        wo_sb = consts.tile([128, 1024], BF, tag="wo")

        # ---- persistent intermediates (memsets BEFORE any DMA trigger on
        # gpsimd: zero/ones planes gate the first sims and AVs, and the
        # gpsimd queue would otherwise run them after its DMA triggers) ----
        qTpad = sbuf.tile([128, 2, BN], BF, tag="qTpad")
        kT = sbuf.tile([128, BN], BF, tag="kT")
        # v blocks [kpos, head, 64 dims | 64 ones]; ones columns make AV
        # emit Sum(e) replicated on pav rows 64:128.
        vb = sbuf.tile([128, 32, 2, 128], BF, tag="vb")
        outT = sbuf.tile([128, BN], BF, tag="outT")
        nc.gpsimd.memset(qTpad[64:128, 0, :], 0.0)
        nc.gpsimd.memset(qTpad[0:64, 1, :], 0.0)
        nc.gpsimd.memset(vb[:, 0:16, 0, 64:128], 1.0)
        nc.gpsimd.memset(vb[:, 0:16, 1, 64:128], 1.0)
        nc.gpsimd.memset(vb[:, 16:32, 0, 64:128], 1.0)
        nc.gpsimd.memset(vb[:, 16:32, 1, 64:128], 1.0)

        def xt_band(eng, j):
            eng.dma_start(xt_sb[:, j], xt_d[j])

        # ---- DMA schedule. Big descriptors; rings drain roughly in
        # trigger order, so issue strictly by need-time. sync carries the
        # critical path (wq -> xt0 -> wk -> xt bands); scalar carries the
        # small urgent tables; gpsimd (busy with memsets first) gets the
        # late tables.
        nc.sync.dma_start(wq_sb[:], wq_d[:].rearrange("p (k m) -> p k m", k=8))
        nc.sync.dma_start(xt_sb[:, 0, 0:4], xt_d[0][:, 0:4])
        nc.sync.dma_start(xt_sb[:, 0, 4:8], xt_d[0][:, 4:8])
        nc.sync.dma_start(wk_sb[:], wk_d[:].rearrange("p (k m) -> p k m", k=8))
        xt_band(nc.sync, 1)
        xt_band(nc.sync, 2)
        xt_band(nc.sync, 3)
        xt_band(nc.sync, 4)
        xt_band(nc.sync, 5)
        xt_band(nc.sync, 6)
        xt_band(nc.sync, 7)

        nc.scalar.dma_start(cosq_sb[:, 0:QT], cosq_d[:, 0:QT])
        nc.scalar.dma_start(sinq_sb[:, 0:QT], sinq_d[:, 0:QT])
        nc.scalar.dma_start(wv_sb[:], wv_d[:].rearrange("p (k m) -> p k m", k=8))
        nc.scalar.dma_start(cosk_sb[:, 0:QT], cosk_d[:, 0:QT])
        nc.scalar.dma_start(sink_sb[:, 0:QT], sink_d[:, 0:QT])
        nc.scalar.dma_start(iden_sb[:], iden_d[:])
        nc.scalar.dma_start(tri_sb[:], tri_d[:])
        nc.scalar.dma_start(cosq_sb[:, QT:1024], cosq_d[:, QT:1024])
        nc.scalar.dma_start(sinq_sb[:, QT:1024], sinq_d[:, QT:1024])
        nc.scalar.dma_start(cosk_sb[:, QT:1024], cosk_d[:, QT:1024])
        nc.scalar.dma_start(sink_sb[:, QT:1024], sink_d[:, QT:1024])
        nc.scalar.dma_start(wo_sb[:], wo_d[:])

        nc.gpsimd.dma_start(cosq_sb[:, 1024:2048], cosq_d[:, 1024:2048])
        nc.gpsimd.dma_start(sinq_sb[:, 1024:2048], sinq_d[:, 1024:2048])
        nc.gpsimd.dma_start(cosk_sb[:, 1024:2048], cosk_d[:, 1024:2048])
        nc.gpsimd.dma_start(sink_sb[:, 1024:2048], sink_d[:, 1024:2048])
        nc.gpsimd.dma_start(cosq_sb[:, 2048:3072], cosq_d[:, 2048:3072])
        nc.gpsimd.dma_start(sinq_sb[:, 2048:3072], sinq_d[:, 2048:3072])
        nc.gpsimd.dma_start(cosq_sb[:, 3072:4096], cosq_d[:, 3072:4096])
        nc.gpsimd.dma_start(sinq_sb[:, 3072:4096], sinq_d[:, 3072:4096])

        # ---- emission units ----

        def proj_chain(w_sb, j):
            """8-step accumulation chain for band j into one 1-bank slot."""
            holder = {}

            def step(kc, holder=holder):
                if kc == 0:
                    holder["raw"] = ps_fill.tile(
                        [128, QT], F32, tag="fill", name="chain"
                    )
                nc.tensor.matmul(
                    holder["raw"],
                    w_sb[:, kc, :],
                    xt_sb[:, j, kc, :],
                    start=(kc == 0),
                    stop=(kc == 7),
                )

            return holder, [(1, lambda kc=kc: step(kc)) for kc in range(8)]

        # head dims are host-permuted into interleaved rotate-half pairs
        # (2f <- f, 2f+1 <- f+32), so rotate_half is an adjacent-pair
        # partition swap: one DVE stream_shuffle
        SWAP_MASK = [i ^ 1 for i in range(32)]

        def rope_unit(holder, j, cos_sb, sin_sb, is_q):
            """cast raw (PSUM f32) to SBUF bf16, then rope via shuffle +
            two bf16 muls + add, all at 2x DVE rate."""
            jc = slice(j * QT, (j + 1) * QT)
            jq = j % NQT_B
            tc_ = jc if is_q else slice(jq * QT, (jq + 1) * QT)

            def cast():
                raws = rawp.tile([128, QT], BF, tag="raws", name="raws")
                nc.vector.tensor_copy(raws[:], holder["raw"])
                holder["raws"] = raws

            def run():
                raws = holder["raws"]
                t2r = work.tile([128, QT], BF, tag="t2r")
                nc.vector.stream_shuffle(t2r[:], raws[:], SWAP_MASK)
                t1 = work.tile([128, QT], BF, tag="t1")
                nc.vector.tensor_mul(t1[:], raws[:], cos_sb[:, tc_])
                t2 = work.tile([128, QT], BF, tag="t2")
                nc.vector.tensor_mul(t2[:], t2r[:], sin_sb[:, tc_])
                if is_q:
                    nc.vector.tensor_add(
                        qTpad[0:64, 0, jc], t1[0:64, :], t2[0:64, :]
                    )
                    nc.vector.tensor_add(
                        qTpad[64:128, 1, jc], t1[64:128, :], t2[64:128, :]
                    )
                else:
                    nc.vector.tensor_add(kT[:, jc], t1[:], t2[:])

            return [(0, cast), (0, run)]

        def vt_units(j):
            """vT chain + transpose/scatter into vb for band j."""
            holder, steps = proj_chain(wv_sb, j)
            units = list(steps)

            def copyout():
                vtsb = work.tile([128, QT], BF, tag="vtsb", bufs=2)
                nc.vector.tensor_copy(vtsb[:], holder["raw"])
                holder["vtsb"] = vtsb

            units.append((0, copyout))

            def tr(u, j=j):
                vtsb = holder["vtsb"]
                tp = ps_fill.tile([128, 128], BF, tag="fill", name="tp")
                nc.tensor.transpose(
                    tp[:, 0:128],
                    vtsb[:, u * 128:(u + 1) * 128],
                    iden_sb[:],
                )
                rc = j * 4 + u
                nc.vector.tensor_copy(
                    vb[:, rc, :, 0:64],
                    tp[:].rearrange("p (h d) -> p h d", h=2),
                )

            units.extend((1, lambda u=u: tr(u)) for u in range(4))
            return units

        def j_units(j):
            """All projection work for band j. The PSUM->SBUF casts are
            emitted immediately after their chain (ahead of the rope
            arithmetic) so the fill slots recycle fast and the next chain
            never waits on the vector queue's rope backlog."""
            qh, qsteps = proj_chain(wq_sb, j)
            cast_q, run_q = rope_unit(qh, j, cosq_sb, sinq_sb, True)
            kh, ksteps = proj_chain(wk_sb, j)
            cast_k, run_k = rope_unit(kh, j, cosk_sb, sink_sb, False)
            vsteps = vt_units(j)
            units = []
            units.extend(qsteps)
            units.append(cast_q)
            units.extend(ksteps)
            units.append(cast_k)
            units.append(run_q)
            units.extend(vsteps[:9])       # v chain + copyout
            units.append(run_k)
            units.extend(vsteps[9:])       # transposes
            return units

        def outproj(m, endgame=False):
            mrows = slice(m * 128, (m + 1) * 128)
            # endgame blocks (after the last AV) borrow the now-dead
            # sim/av PSUM slots so 3 blocks can be in flight instead of 1
            if endgame:
                # NOT the "av" slots - pav is still being read by the
                # remaining tail chunks
                po0 = ps_sim.tile([128, QT], F32, tag="sim", name="po0")
                po1 = ps_fill.tile([128, QT], F32, tag="fill", name="po1")
            else:
                po0 = ps_fill.tile([128, QT], F32, tag="fill", name="po0")
                po1 = ps_fill.tile([128, QT], F32, tag="fill", name="po1")
            nc.tensor.matmul(
                po0[:], outT[:, mrows], wo_sb[:, 0:QT], start=True, stop=True
            )
            nc.tensor.matmul(
                po1[:], outT[:, mrows], wo_sb[:, QT:1024], start=True, stop=True
            )
            # stage to SBUF bf16 (PSUM is not DMA-able). While ANY
            # attention remains (m < 28) scalar must stay pure-exp, so
            # both casts ride vector; the inline endgame blocks (m >= 28,
            # attention fully done) split the pair across both engines.
            ot = work.tile([128, 1024], BF, tag="ot")
            if m >= 28:
                nc.scalar.copy(ot[:, 0:QT], po0[:])
            else:
                nc.vector.tensor_copy(ot[:, 0:QT], po0[:])
            nc.vector.tensor_copy(ot[:, QT:1024], po1[:])
            oeng = nc.gpsimd if m % 2 == 0 and m < 24 else nc.sync
            oeng.dma_start(out_d[mrows, :], ot[:])

        def attn_units(b):
            """Yields ("kc", closure) and ("tail", t, closure)."""
            base = b * N
            for t in range(NQT_B):
                qcols = slice(base + t * QT, base + (t + 1) * QT)
                nkc = 4 * (t + 1)
                pav = [None, None]
                yield ("gate", 4 * b + t)

                pend_av = [None]

                def emit_av(kc, e, lo, b=b, pav=pav, nkc=nkc):
                    if kc == 0:
                        pav[0] = ps_av.tile([128, QT], F32, tag="av", name="pav0")
                        pav[1] = ps_av.tile([128, QT], F32, tag="av", name="pav1")
                    for h in range(2):
                        nc.tensor.matmul(
                            pav[h][:, lo:],
                            vb[:, b * 16 + kc, h, :],
                            e[:, h, lo:],
                            start=(kc == 0),
                            stop=(kc == nkc - 1),
                        )

                def kc_block(kc, t=t, b=b, base=base, pend_av=pend_av):
                    off = kc * KC - t * QT
                    lo = max(0, off)
                    kcols = slice(base + kc * KC, base + (kc + 1) * KC)
                    qsub = slice(base + t * QT + lo, base + (t + 1) * QT)
                    ps_s = ps_sim.tile([128, 2, QT], F32, tag="sim", name="sim")
                    for h in range(2):
                        nc.tensor.matmul(
                            ps_s[:, h, lo:],
                            kT[:, kcols],
                            qTpad[:, h, qsub],
                            start=True,
                            stop=True,
                        )
                    e = epool.tile([128, 2, QT], BF, tag="e", name="e")
                    nc.scalar.activation(e[:, :, lo:], ps_s[:, :, lo:], AF.Exp)
                    if off >= 0:
                        # only the 128-col diagonal window is mixed; the
                        # rest of the row is fully unmasked
                        hi = lo + KC
                        nc.gpsimd.tensor_mul(
                            e[:, 0, lo:hi], e[:, 0, lo:hi], tri_sb[:]
                        )
                        nc.gpsimd.tensor_mul(
                            e[:, 1, lo:hi], e[:, 1, lo:hi], tri_sb[:]
                        )
                    # software pipeline: emit the PREVIOUS kc's AV here so
                    # this kc's sims sit between exp(kc-1) and AV(kc-1) in
                    # the PE queue, hiding the activation latency
                    if pend_av[0] is not None:
                        emit_av(*pend_av[0])
                    pend_av[0] = (kc, e, lo)

                def flush_av(pend_av=pend_av):
                    emit_av(*pend_av[0])
                    pend_av[0] = None

                for kc in range(nkc):
                    yield ("kc", lambda kc=kc, f=kc_block: f(kc))
                yield ("kc", lambda: flush_av())

                last = (b == 1 and t == NQT_B - 1)

                def tail_chunk(c, nchunks, qcols=qcols, pav=pav):
                    cw = QT // nchunks
                    cs = slice(c * cw, (c + 1) * cw)
                    ocs = slice(qcols.start + c * cw, qcols.start + (c + 1) * cw)
                    for h in range(2):
                        hp = slice(64 * h, 64 * h + 64)
                        lnt = rpool.tile([64, cw], F32, tag="lnt", name="lnt")
                        nc.scalar.activation(lnt[:], pav[h][64:128, cs], AF.Ln)
                        rec = rpool.tile([64, cw], BF, tag="rec", name="rec")
                        nc.scalar.activation(rec[:], lnt[:], AF.Exp, scale=-1.0)
                        nc.vector.tensor_mul(
                            outT[hp, ocs], pav[h][0:64, cs], rec[:]
                        )

                yield ("tail", b, t, last, tail_chunk)

        # ---- prologue: band 0 only, then one unified schedule:
        # attention qtiles (both batches, in order) interleaved with the
        # remaining per-j projection units (need-ordered, matching DMA
        # arrival) and, as soon as each qtile's tail lands, its
        # out-projection blocks. ----
        for _, f in j_units(0):
            f()

        pending = []
        j_end = {0: 0}
        for j in range(1, 8):
            pending.extend(j_units(j))
            j_end[j] = len(pending)
        fi = 0

        def attn_all():
            yield from attn_units(0)
            yield from attn_units(1)

        gate_now = [0]
        for item in attn_all():
            if item[0] == "gate":
                # qtile (b,t) reads qTpad/kT/vb written by j-units up to
                # j = 4b+t; those must be EMITTED before the qtile's
                # instructions or the reads get no dependency edge
                gate_now[0] = item[1]
                need = j_end[item[1]]
                while fi < need:
                    pending[fi][1]()
                    fi += 1
            elif item[0] == "kc":
                item[1]()
                # hold back out-projection work (appended past nproj) so
                # the filler-starved final qtiles stay PE-dense: b1t3
                # (gate 7) gets everything, b1t2 keeps 12 pe-units in
                # reserve, earlier tiles keep 16
                if gate_now[0] >= 7:
                    limit = len(pending)
                elif gate_now[0] == 6:
                    limit = len(pending) - 6
                else:
                    limit = max(
                        j_end[7], fi + max(0, len(pending) - fi - 16)
                    )
                budget = 2
                while fi < min(limit, len(pending)) and (
                    budget > 0 or pending[fi][0] == 0
                ):
                    pe, f = pending[fi]
                    f()
                    budget -= pe
                    fi += 1
            else:
                _, b_, t_, last_, tailf = item
                if last_:
                    # final qtile: run each 128-token chunk's
                    # normalization one chunk AHEAD of its out-projection
                    # so the norm chain latency never idles the PE; drain
                    # held-back blocks between chunks to keep it warm
                    m0 = 16 * b_ + 4 * t_
                    tailf(0, 4)
                    for c in range(4):
                        if c + 1 < 4:
                            tailf(c + 1, 4)
                        outproj(m0 + c, endgame=True)
                        budget = 3
                        while fi < len(pending) and budget > 0:
                            pe, f = pending[fi]
                            f()
                            budget -= pe
                            fi += 1
                else:
                    tailf(0, 1)
                    # filler burst: give the PE queue extra work to chew
                    # on while the tail chain releases the AV banks (but
                    # respect the endgame hold-back)
                    budget = 3
                    hold = 6 if gate_now[0] >= 6 else 16
                    while fi < len(pending) - hold and budget > 0:
                        pe, f = pending[fi]
                        f()
                        budget -= pe
                        fi += 1
                    for m in range(16 * b_ + 4 * t_, 16 * b_ + 4 * t_ + 4):
                        pending.append((2, lambda m=m: outproj(m)))
        while fi < len(pending):
            pending[fi][1]()
            fi += 1

    _split_excess_waits(nc)
    return nc


_PROGRAM = None


def _get_program():
    global _PROGRAM
    if _PROGRAM is None:
        _PROGRAM = _build_program()
    return _PROGRAM


def _host_prep(x, pos_emb, gamma, Wq, Wkv, Wo):
    """Build the per-core input maps."""
    xf = np.ascontiguousarray(x.reshape(BN, D))
    xT = np.ascontiguousarray(xf.T).astype(BF16)        # [1024, 4096]
    # [jband, part, kc, col]: band j holds tokens j*512:(j+1)*512 for all
    # 8 feature chunks -> per-partition 8KB contiguous DMA lines
    xt = np.ascontiguousarray(
        xT.reshape(8, 128, 8, QT).transpose(2, 1, 0, 3)
    )

    r = 1.0 / np.maximum(
        np.linalg.norm(xf.astype(np.float64), axis=1).astype(np.float32)
        * (D ** -0.5),
        EPS,
    )

    fr = pos_emb[0, 0, :, :32].astype(np.float32)        # [N, 32]
    cos_t = np.cos(fr).T                                 # [32, N]
    sin_t = np.sin(fr).T
    # interleaved-pair head-dim order: partition p holds old dim
    # perm64[p%64] of its head; pair (2f, 2f+1) <- old (f, f+32).
    # rope there: out = t*cos + swap_adj(t)*sin with sign -1 on even p.
    pmod = np.arange(128) % 64
    fidx = pmod // 2                                     # freq index per row
    cos128 = cos_t[fidx, :].astype(np.float32)           # [128, N]
    sgn = np.where(np.arange(128) % 2 == 0, -1.0, 1.0).astype(np.float32)
    sin128s = (sin_t[fidx, :] * sgn[:, None]).astype(np.float32)
    colpos = np.arange(BN) % N
    rq = (r * SCALE).astype(np.float32)
    cosq = (cos128[:, colpos] * rq[None, :]).astype(BF16)
    sinq = (sin128s[:, colpos] * rq[None, :]).astype(BF16)
    cosk = cos128.astype(BF16)
    sink = sin128s.astype(BF16)
    # old-dim index for interleaved position d (within a 64-dim head)
    perm64 = np.empty(64, np.int64)
    perm64[0::2] = np.arange(32)
    perm64[1::2] = np.arange(32) + 32

    iden = np.eye(128, dtype=np.float32).astype(BF16)

    # causal 0/1 triangle for the 128-col diagonal window: keep iff p <= c
    p = np.arange(128)[:, None]
    c = np.arange(128)[None, :]
    tri = (p <= c).astype(BF16)

    def permute_heads(w):  # apply perm64 within each 64-col head block
        wh = w.reshape(w.shape[0], H, 64)
        return wh[:, :, perm64].reshape(w.shape[0], INNER)

    Wq_s = permute_heads(
        (gamma[:, None].astype(np.float32) * Wq)
    ).astype(BF16)
    Wk = permute_heads(Wkv[:, :INNER].astype(np.float32)).astype(BF16)
    Wv = Wkv[:, INNER:].astype(BF16)
    Wo_b = Wo.astype(BF16)

    def warrange(w):  # [1024, 128] -> [128, 1024] with [p, kc*128+m]
        return np.ascontiguousarray(
            w.reshape(8, 128, 128).transpose(1, 0, 2).reshape(128, 1024)
        )

    in_maps = []
    for c_ in range(NCORES):
        sl = slice(128 * c_, 128 * (c_ + 1))
        in_maps.append(
            {
                "xt": xt,
                "wq": warrange(Wq_s[:, sl]),
                "wk": warrange(Wk[:, sl]),
                "wv": warrange(Wv[:, sl]),
                "wo": np.ascontiguousarray(Wo_b[sl, :]),
                "cosq": cosq,
                "sinq": sinq,
                "cosk": cosk,
                "sink": sink,
                "iden": iden,
                "tri": tri,
            }
        )
    return in_maps


def run(inputs, trace=False, trace_kwargs=None):
    nc = _get_program()
    in_maps = _host_prep(
        np.asarray(inputs["x"]),
        np.asarray(inputs["pos_emb"]),
        np.asarray(inputs["gamma"]),
        np.asarray(inputs["Wq"]),
        np.asarray(inputs["Wkv"]),
        np.asarray(inputs["Wo"]),
    )
    res = run_bass_kernel_spmd(
        nc,
        in_maps,
        list(range(NCORES)),
        trace=trace,
        trace_kwargs=trace_kwargs or {},
    )
    out = np.zeros((BN, D), np.float32)
    for c in range(NCORES):
        out += res.results[c]["out"].astype(np.float32)
    out += np.asarray(inputs["bo"]).astype(np.float32)[None, :]
    out = out.reshape(B, N, D).astype(np.float32)
    return out, res


def kernel(**inputs):
    out, _ = run(inputs, trace=False)
    return out
